# revision 12
# baseline (speedup 1.0000x reference)
"""Trainium2 Bass kernel for nn_AEGConv2d (8 NeuronCores, SPMD).

Problem: out = sigmoid(aeg(x, weight)) * (conv2d(x, conv_w) + conv_b)
  x: (4, 32, 64, 64) f32, weight/conv_w: (64, 32, 3, 3), conv_b: (64,)
  stride=1, padding=1.

The AEG recurrence unrolls to res = sum_k A_k(px) * B_k(cout,cin) per
pixel-parity class s=(i+j)%2, where A_k = x_k * C_{sigma(s,k)} with the
suffix chain C_L over the opposite-class taps, and B_k a host-side
weight product.  The whole AEG conv is a 288-deep matmul per parity.

Sharding: 8 cores = 4 images x 2 row-halves.  No collectives.

Per-core device schedule (v3):
- XPC [plane1 | plane1-shifted]: every chain tap is a single both-grid
  32-partition view.  Chain muls write into the 128-partition multiplier
  stacks M_s = [C2, C3, C4, C1] (C2/C4 direct, C3/C4^s1 via a base-0
  scratch + off-path Pool/ACT copy, C1 an off-path raw-tap copy), which
  double as (a) in1 of one wide 128p "extras" mul per parity with the
  host-packed tap stacks TE0/TE1 and (b) a K=128 aeg-matmul rhs:
  M0 rows = [A_5,A_3,A_1,A_7], M1 = [A_6,A_4,A_2,A_8],
  E0 = [A_4,A_2,A_0,A_6], E1 = [A_5,A_3,A_1,A_7].
- DVE total: 9 muls + 4 epilogue STTs.  A_8^s0 rides the conv matmul
  (braw columns); A_0^s1 gets its own 32p mul + K=32 matmul.
- Matmuls: per quadrant (s,t) 3x conv K=96 + 2-3 aeg; quadrants close
  s1-first and each (s,t) output block DMAs right after its STT.
"""

import numpy as np
import ml_dtypes

import concourse.bacc as bacc
import concourse.bass as bass
import concourse.mybir as mybir
import concourse.tile as tile
from concourse.bass_utils import run_bass_kernel_spmd

F32 = mybir.dt.float32
BF16 = mybir.dt.bfloat16

N, CIN, H, W = 4, 32, 64, 64
COUT, KK = 64, 3
PAD = 1
OH, OW = 32, 64          # per-core output rows x cols
ROWS, COLS = 34, 66      # per-core padded slab
PLP = 34                 # plane row pitch
PLSZ = PLP * ROWS        # 1156 elements per plane per cin
N_CORES = 8

# chain taps (suffix products of the opposite-parity class), low level first:
# C1^s0=x7, C2=x5*C1, C3=x3*C2, C4=x1*C3 ; s1: x8, x6, x4, x2
CHAIN = {0: [7, 5, 3, 1], 1: [8, 6, 4, 2]}
# M-stack group layout is [C2, C3, C4, C1]; row tap identities:
M_TAPS = {0: [5, 3, 1, 7], 1: [6, 4, 2, 8]}
# TE row order multiplies [C2, C3, C4, C1]:
E_TAPS = {0: [4, 2, 0, 6], 1: [5, 3, 1, 7]}

_last_results = None  # stash for test.py (exec_time_ns etc.)


def _fview(base_ap, off, dims):
    """View with the same partition dim as base_ap but custom free dims."""
    return bass.AP(
        tensor=base_ap.tensor,
        offset=base_ap.offset + off,
        ap=[base_ap.ap[0]] + dims,
    )


def build_nc():
    nc = bacc.Bacc(None, target_bir_lowering=False)
    xpc_d = nc.declare_dram_parameter("xpc", [CIN, 2 * PLSZ], BF16, isOutput=False)
    xa_d = nc.declare_dram_parameter("xa", [96, PLSZ], BF16, isOutput=False)
    xb_d = nc.declare_dram_parameter("xb", [96, PLSZ], BF16, isOutput=False)
    te0_d = nc.declare_dram_parameter("te0", [128, 1024], BF16, isOutput=False)
    te1_d = nc.declare_dram_parameter("te1", [128, 1024], BF16, isOutput=False)
    wallc_d = nc.declare_dram_parameter("wallc", [96, 448], BF16, isOutput=False)
    walla_d = nc.declare_dram_parameter("walla", [128, 320], BF16, isOutput=False)
    bias_d = nc.declare_dram_parameter("bias", [COUT, 1], F32, isOutput=False)
    out_d = nc.declare_dram_parameter("out", [4, COUT, 512], BF16, isOutput=True)

    with tile.TileContext(nc) as tc:
        with (
            tc.tile_pool(name="big", bufs=1) as big,
            tc.tile_pool(name="sig", bufs=4) as sigp,
            tc.tile_pool(name="psum", bufs=1, space="PSUM") as pp,
        ):
            # --- input DMAs: first-needed tensors first per queue.
            XPC = big.tile([CIN, 2 * PLSZ], BF16)
            nc.sync.dma_start(out=XPC[:, :], in_=xpc_d[:, :])
            XAB = {}
            xb_t = big.tile([96, PLSZ], BF16, tag="xab1", name="xb_t")
            nc.scalar.dma_start(out=xb_t[:, :], in_=xb_d[:, :])
            XAB[1] = xb_t
            xa_t = big.tile([96, PLSZ], BF16, tag="xab0", name="xa_t")
            nc.sync.dma_start(out=xa_t[:, :], in_=xa_d[:, :])
            XAB[0] = xa_t
            wallc = big.tile([96, 448], BF16)
            nc.gpsimd.dma_start(out=wallc[:, :], in_=wallc_d[:, :])
            TE = {}
            for s, dram, eng in ((1, te1_d, nc.scalar), (0, te0_d, nc.sync)):
                t = big.tile([128, 2, 16, 32], BF16, tag=f"te{s}", name=f"TE{s}")
                eng.dma_start(out=t[:, :, :, :], in_=dram[:, :])
                TE[s] = t
            walla = big.tile([128, 320], BF16)
            nc.scalar.dma_start(out=walla[:, :], in_=walla_d[:, :])
            bias_t = big.tile([COUT, 1], F32)
            nc.gpsimd.dma_start(out=bias_t[:, :], in_=bias_d[:, :])

            M, E = {}, {}
            for s in (0, 1):
                M[s] = big.tile([128, 2, 16, 32], BF16, tag=f"m{s}", name=f"M{s}")
                E[s] = big.tile([128, 2, 16, 32], BF16, tag=f"e{s}", name=f"E{s}")
            CH0 = big.tile([32, 2, 16, 32], BF16)   # C3^s0 scratch
            CH1 = big.tile([32, 2, 16, 32], BF16)   # C3^s1 scratch
            CH1B = big.tile([32, 2, 16, 32], BF16)  # C4^s1 scratch
            A0T = big.tile([32, 2, 16, 32], BF16)
            out_sb = big.tile([COUT, 4, 16, 32], BF16)
            scratch = big.tile([1, 8], F32)

            def xview(k, s):
                """Both-grid (32,[2,16,32]) base-0 view of tap k in XPC."""
                ki, kj = divmod(k, 3)
                assert (s + ki + kj) % 2 == 1, "chain/XPC taps live on plane 1"
                off = []
                for t in (0, 1):
                    m = ((s ^ t) + kj) // 2
                    off.append(m * PLSZ + (t + ki) * PLP)
                return _fview(XPC[0:32, :], off[0],
                              [[off[1] - off[0], 2], [2 * PLP, 16], [1, 32]])

            def mg(s, g):
                return M[s][32 * g : 32 * g + 32, :, :, :]

            # --- off-path copies into M slots ---
            # Pool: C1^s1 raw tap, then C3^s1 from scratch
            nc.gpsimd.tensor_copy(mg(1, 3), xview(8, 1))
            # ACT: C1^s0 raw tap
            nc.scalar.activation(mg(0, 3), xview(7, 0),
                                 mybir.ActivationFunctionType.Copy)

            # --- DVE: 6 chain muls + A0^s1 + 2 wide extras ---
            # touch: absorb the XPC DMA wait
            nc.vector.tensor_copy(scratch[0:1, 0:1], XPC[0:1, 0:1])
            ch1v = CH1[:, :, :, :]
            ch1bv = CH1B[:, :, :, :]
            ch0v = CH0[:, :, :, :]
            # s1 chain: C2 -> M1[g0] direct; C3 -> CH1; C4 -> CH1B
            nc.vector.tensor_mul(mg(1, 0), xview(6, 1), xview(8, 1))
            nc.vector.tensor_mul(ch1v, xview(4, 1), M[1][0:32, :, :, :])
            nc.vector.tensor_mul(ch1bv, xview(2, 1), ch1v)
            nc.gpsimd.tensor_copy(mg(1, 1), ch1v)       # C3^s1 -> M1[g1]
            nc.scalar.activation(mg(1, 2), ch1bv,       # C4^s1 -> M1[g2]
                                 mybir.ActivationFunctionType.Copy)
            # A_0^s1 = x0 * C4^s1
            nc.vector.tensor_mul(A0T[:, :, :, :], xview(0, 1), ch1bv)
            # s0 chain: C2 -> M0[g0] direct; C3 -> CH0; C4 -> M0[g2] direct
            nc.vector.tensor_mul(mg(0, 0), xview(5, 0), xview(7, 0))
            # E1 = TE1 * M1 (M1 complete: g0 direct, g1/g2/g3 copies)
            nc.vector.tensor_copy(scratch[0:1, 0:1], TE[1][0:1, 0:1, 0:1, 0:1])
            nc.vector.tensor_copy(scratch[0:1, 1:2], M[1][32:33, 0:1, 0:1, 0:1])
            nc.vector.tensor_copy(scratch[0:1, 2:3], M[1][64:65, 0:1, 0:1, 0:1])
            nc.vector.tensor_mul(E[1][:, :, :, :], TE[1][:, :, :, :], M[1][:, :, :, :])
            nc.vector.tensor_mul(ch0v, xview(3, 0), M[0][0:32, :, :, :])
            nc.scalar.activation(mg(0, 1), ch0v,        # C3^s0 -> M0[g1]
                                 mybir.ActivationFunctionType.Copy)
            nc.vector.tensor_mul(mg(0, 2), xview(1, 0), ch0v)
            # E0 = TE0 * M0
            nc.vector.tensor_copy(scratch[0:1, 0:1], TE[0][0:1, 0:1, 0:1, 0:1])
            nc.vector.tensor_copy(scratch[0:1, 1:2], M[0][32:33, 0:1, 0:1, 0:1])
            nc.vector.tensor_copy(scratch[0:1, 2:3], M[0][96:97, 0:1, 0:1, 0:1])
            nc.vector.tensor_mul(E[0][:, :, :, :], TE[0][:, :, :, :], M[0][:, :, :, :])

            # --- matmuls ---
            def convgrid(kj, s, t):
                """(96, 16,32) K=96 conv rhs: kernel-column kj, grid t."""
                c = (s + kj) % 2
                m = ((s ^ t) + kj) // 2
                off = t * PLP + m
                return _fview(XAB[c][:, :], off, [[2 * PLP, 16], [1, 32]])

            psq = {}

            def emit_conv(s, t):
                ps = pp.tile([128, 16, 32], F32, tag=f"ps{s}{t}", name=f"ps{s}{t}")
                psq[(s, t)] = ps
                if s == 1:
                    # conv-only, M=64 into rows 64:128; aeg region started
                    # separately by the first aeg matmul (start=True there).
                    for kj in range(3):
                        nc.tensor.matmul(
                            ps[64:128, :, :],
                            wallc[:, 64 * kj : 64 * kj + 64],
                            convgrid(kj, s, t),
                            start=(kj == 0), stop=False,
                            skip_group_check=True,
                        )
                else:
                    # kj2 first with braw columns (A_8^s0), M=128, resets both
                    nc.tensor.matmul(
                        ps[:, :, :], wallc[:, 320:448], convgrid(2, s, t),
                        start=True, stop=False, skip_group_check=True,
                    )
                    for kj in (0, 1):
                        nc.tensor.matmul(
                            ps[64:128, :, :],
                            wallc[:, 192 + 64 * kj : 256 + 64 * kj],
                            convgrid(kj, s, t),
                            start=False, stop=False, skip_group_check=True,
                        )

            def emit_aeg(s, t, which, start, stop):
                ps = psq[(s, t)]
                if which == "m":
                    lh = walla[:, 64 * (2 * s) : 64 * (2 * s) + 64]
                    rh = M[s][:, t, :, :]
                elif which == "e":
                    lh = walla[:, 64 * (2 * s + 1) : 64 * (2 * s + 1) + 64]
                    rh = E[s][:, t, :, :]
                else:  # a0 (s=1 only)
                    lh = walla[0:32, 256:320]
                    rh = A0T[:, t, :, :]
                nc.tensor.matmul(
                    ps[0:64, :, :], lh, rh,
                    start=start, stop=stop, skip_group_check=True,
                )

            emit_conv(1, 0)
            emit_conv(1, 1)
            emit_aeg(1, 0, "m", True, False)
            emit_aeg(1, 1, "m", True, False)
            emit_aeg(1, 0, "e", False, False)
            emit_aeg(1, 1, "e", False, False)
            emit_aeg(1, 0, "a0", False, True)
            emit_aeg(1, 1, "a0", False, True)
            emit_conv(0, 0)
            emit_conv(0, 1)
            emit_aeg(0, 0, "m", False, False)
            emit_aeg(0, 1, "m", False, False)
            emit_aeg(0, 0, "e", False, True)
            emit_aeg(0, 1, "e", False, True)

            # --- epilogue: sigmoid(aeg) * (conv + bias), per-quadrant DMA ---
            def emit_epi(s, t, eng):
                ps = psq[(s, t)]
                sig = sigp.tile([64, 16, 32], F32)
                nc.scalar.activation(
                    sig[:, :, :], ps[0:64, :, :],
                    mybir.ActivationFunctionType.Sigmoid,
                )
                nc.vector.tensor_copy(scratch[0:1, 0:1], sig[0:1, 0:1, 0:1])
                b = 2 * s + t
                nc.vector.scalar_tensor_tensor(
                    out=out_sb[:, b, :, :],
                    in0=ps[64:128, :, :],
                    scalar=bias_t[:, 0:1],
                    in1=sig[:, :, :],
                    op0=mybir.AluOpType.add,
                    op1=mybir.AluOpType.mult,
                )
                eng.dma_start(
                    out=out_d[b, :, :],
                    in_=out_sb[:, b, :, :],
                )

            emit_epi(1, 0, nc.sync)
            emit_epi(1, 1, nc.scalar)
            emit_epi(0, 0, nc.sync)
            emit_epi(0, 1, nc.scalar)
    nc.finalize()
    return nc


def _host_prep(x, weight, conv_w, conv_b):
    """Shard + pack per-core inputs (bf16 parity planes + weight products)."""
    bf16 = ml_dtypes.bfloat16
    xp = np.pad(np.ascontiguousarray(x, np.float32),
                ((0, 0), (0, 0), (PAD, PAD), (PAD, PAD)))
    kflat = weight.reshape(COUT, CIN, 9).transpose(2, 0, 1)  # (9, cout, cin)
    B = np.zeros((2, 9, COUT, CIN), np.float32)
    for s in (0, 1):
        suf = np.ones((COUT, CIN), np.float32)
        for k in range(8, -1, -1):
            B[s, k] = kflat[k] * suf
            if k % 2 == s:
                suf = suf * kflat[k]
    wc_k = conv_w.reshape(COUT, CIN, 9)  # (cout, cin, k)

    # conv lhsT [96, 448]: s1 kj0..2 (M=64) | s0 kj0, kj1 (M=64) |
    # s0 kj2 [braw | conv] (M=128)
    wallc = np.zeros((96, 448), np.float32)
    for kj in range(3):
        for ki in range(3):
            k = ki * 3 + kj
            blk = slice(32 * ki, 32 * ki + 32)
            wallc[blk, 64 * kj : 64 * kj + 64] = wc_k[:, :, k].T          # s1
            if kj < 2:
                wallc[blk, 192 + 64 * kj : 256 + 64 * kj] = wc_k[:, :, k].T
            else:
                wallc[blk, 384:448] = wc_k[:, :, k].T
    wallc[64:96, 320:384] = B[0, 8].T  # braw: A_8^s0 on the kj2 rhs rows

    # aeg lhsT: bM0 | bE0 | bM1 | bE1 | bA0
    walla = np.zeros((128, 320), np.float32)
    for s in (0, 1):
        for g, k in enumerate(M_TAPS[s]):
            walla[32 * g : 32 * g + 32, 64 * (2 * s) : 64 * (2 * s) + 64] = B[s, k].T
        for g, k in enumerate(E_TAPS[s]):
            walla[32 * g : 32 * g + 32,
                  64 * (2 * s + 1) : 64 * (2 * s + 1) + 64] = B[s, k].T
    walla[0:32, 256:320] = B[1, 0].T

    wallc_p = wallc.astype(bf16)
    walla_p = walla.astype(bf16)
    bias_p = np.ascontiguousarray(conv_b.reshape(COUT, 1), np.float32)

    in_maps = []
    for core in range(N_CORES):
        n, h = divmod(core, 2)
        slab = xp[n, :, 32 * h : 32 * h + ROWS, :]  # (32, 34, 66) f32
        plane1 = np.zeros((CIN, ROWS, PLP), np.float32)
        for r in range(ROWS):
            b = (1 + r) % 2
            cols = slab[:, r, b::2]
            plane1[:, r, : cols.shape[1]] = cols
        plane0 = np.zeros((CIN, ROWS, PLP), np.float32)
        for r in range(ROWS):
            b = r % 2
            cols = slab[:, r, b::2]
            plane0[:, r, : cols.shape[1]] = cols
        planes = {0: plane0, 1: plane1}
        # XPC: [plane1 | plane1-shifted]
        xpc = np.zeros((CIN, 2, ROWS, PLP), np.float32)
        xpc[:, 0] = plane1
        xpc[:, 1, :, : PLP - 1] = plane1[:, :, 1:]
        xpc_core = np.ascontiguousarray(xpc.reshape(CIN, 2 * PLSZ)).astype(bf16)
        # xa/xb: partition-stacked row-shifted plane sets for conv rhs
        xab = np.zeros((2, 3, CIN, ROWS, PLP), np.float32)
        for c in (0, 1):
            for r in range(3):
                q = (c + r) % 2
                xab[c, r, :, : ROWS - r] = planes[q][:, r:]
        xa_core = np.ascontiguousarray(xab[0].reshape(96, PLSZ)).astype(bf16)
        xb_core = np.ascontiguousarray(xab[1].reshape(96, PLSZ)).astype(bf16)
        # TE tap stacks (tight grid-major (2,16,32) per tap)
        te = np.zeros((2, 4, CIN, 2, 16, 32), np.float32)
        for s in (0, 1):
            for g, k in enumerate(E_TAPS[s]):
                ki, kj = divmod(k, 3)
                for t in (0, 1):
                    te[s, g, :, t] = slab[:, t + ki : t + ki + 32 : 2,
                                          (s ^ t) + kj : (s ^ t) + kj + 64 : 2]
        te0_core = np.ascontiguousarray(te[0].reshape(128, 1024)).astype(bf16)
        te1_core = np.ascontiguousarray(te[1].reshape(128, 1024)).astype(bf16)
        in_maps.append({
            "xpc": xpc_core,
            "xa": xa_core,
            "xb": xb_core,
            "te0": te0_core,
            "te1": te1_core,
            "wallc": wallc_p,
            "walla": walla_p,
            "bias": bias_p,
        })
    return in_maps


_nc_cache = None


def kernel(x, weight, conv_w, conv_b, trace=False):
    global _nc_cache, _last_results
    x = np.asarray(x, np.float32)
    weight = np.asarray(weight, np.float32)
    conv_w = np.asarray(conv_w, np.float32)
    conv_b = np.asarray(conv_b, np.float32)

    if _nc_cache is None:
        _nc_cache = build_nc()
    nc = _nc_cache
    in_maps = _host_prep(x, weight, conv_w, conv_b)
    res = run_bass_kernel_spmd(nc, in_maps, core_ids=list(range(N_CORES)), trace=trace)
    _last_results = res

    out = np.empty((N, COUT, H, W), np.float32)
    for core in range(N_CORES):
        n, h = divmod(core, 2)
        blk = res.results[core]["out"].astype(np.float32).reshape(2, 2, COUT, 16, 32)
        for s in (0, 1):
            for t in (0, 1):
                out[n, :, 32 * h + t : 32 * h + t + 32 : 2,
                    (s ^ t) :: 2] = blk[s, t]
    return out


# revision 17
# speedup vs baseline: 1.0762x; 1.0762x over previous
"""Trainium2 Bass kernel for nn_AEGConv2d (8 NeuronCores, SPMD).

Problem: out = sigmoid(aeg(x, weight)) * (conv2d(x, conv_w) + conv_b)
  x: (4, 32, 64, 64) f32, weight/conv_w: (64, 32, 3, 3), conv_b: (64,)
  stride=1, padding=1.

The AEG recurrence unrolls to res = sum_k A_k(px) * B_k(cout,cin) per
pixel-parity class s=(i+j)%2, where A_k = x_k * C_{sigma(s,k)} with the
suffix chain C_L over the opposite-class taps, and B_k a host-side
weight product.  The whole AEG conv is a 288-deep matmul per parity.

Sharding: 8 cores = 4 images x 2 row-halves.  No collectives.

Per-core device schedule (v3):
- XPC [plane1 | plane1-shifted]: every chain tap is a single both-grid
  32-partition view.  Chain muls write into the 128-partition multiplier
  stacks M_s = [C2, C3, C4, C1] (C2/C4 direct, C3/C4^s1 via a base-0
  scratch + off-path Pool/ACT copy, C1 an off-path raw-tap copy), which
  double as (a) in1 of one wide 128p "extras" mul per parity with the
  host-packed tap stacks TE0/TE1 and (b) a K=128 aeg-matmul rhs:
  M0 rows = [A_5,A_3,A_1,A_7], M1 = [A_6,A_4,A_2,A_8],
  E0 = [A_4,A_2,A_0,A_6], E1 = [A_5,A_3,A_1,A_7].
- DVE total: 9 muls + 4 epilogue STTs.  A_8^s0 rides the conv matmul
  (braw columns); A_0^s1 gets its own 32p mul + K=32 matmul.
- Matmuls: per quadrant (s,t) 3x conv K=96 + 2-3 aeg; quadrants close
  s1-first and each (s,t) output block DMAs right after its STT.
"""

import numpy as np
import ml_dtypes

import concourse.bacc as bacc
import concourse.bass as bass
import concourse.mybir as mybir
import concourse.tile as tile
from concourse.bass_utils import run_bass_kernel_spmd

F32 = mybir.dt.float32
BF16 = mybir.dt.bfloat16

N, CIN, H, W = 4, 32, 64, 64
COUT, KK = 64, 3
PAD = 1
OH, OW = 32, 64          # per-core output rows x cols
ROWS, COLS = 34, 66      # per-core padded slab
PLP = 34                 # plane row pitch
PLSZ = PLP * ROWS        # 1156 elements per plane per cin
N_CORES = 8

# chain taps (suffix products of the opposite-parity class), low level first:
# C1^s0=x7, C2=x5*C1, C3=x3*C2, C4=x1*C3 ; s1: x8, x6, x4, x2
CHAIN = {0: [7, 5, 3, 1], 1: [8, 6, 4, 2]}
# M-stack group layout is [C2, C3, C4, C1]; row tap identities:
M_TAPS = {0: [5, 3, 1, 7], 1: [6, 4, 2, 8]}
# TE row order multiplies [C2, C3, C4, C1]:
E_TAPS = {0: [4, 2, 0, 6], 1: [5, 3, 1, 7]}

_last_results = None  # stash for test.py (exec_time_ns etc.)


def _fview(base_ap, off, dims):
    """View with the same partition dim as base_ap but custom free dims."""
    return bass.AP(
        tensor=base_ap.tensor,
        offset=base_ap.offset + off,
        ap=[base_ap.ap[0]] + dims,
    )


def build_nc():
    nc = bacc.Bacc(None, target_bir_lowering=False)
    # plane-1 pair [unshifted | col-shifted], duplicated so chain muls can
    # read tap views at partition base 0 or 32 to match their in1 M-slot
    # (2-input DVE ops require equal base partitions for both inputs).
    xpc_d = nc.declare_dram_parameter("xpc", [64, 2 * PLSZ], BF16, isOutput=False)
    xa_d = nc.declare_dram_parameter("xa", [96, PLSZ], BF16, isOutput=False)
    xb_d = nc.declare_dram_parameter("xb", [96, PLSZ], BF16, isOutput=False)
    te0_d = nc.declare_dram_parameter("te0", [128, 1024], BF16, isOutput=False)
    te1_d = nc.declare_dram_parameter("te1", [128, 1024], BF16, isOutput=False)
    # host-packed C1 rows (raw x7^s0 / x8^s1 in E-grid layout) DMA'd
    # straight into M[s][96:128]
    ms0_d = nc.declare_dram_parameter("ms0", [32, 1024], BF16, isOutput=False)
    ms1_d = nc.declare_dram_parameter("ms1", [32, 1024], BF16, isOutput=False)
    wallc_d = nc.declare_dram_parameter("wallc", [96, 448], BF16, isOutput=False)
    walla_d = nc.declare_dram_parameter("walla", [128, 320], BF16, isOutput=False)
    bias_d = nc.declare_dram_parameter("bias", [COUT, 1], F32, isOutput=False)
    out_d = nc.declare_dram_parameter("out", [4, COUT, 512], BF16, isOutput=True)

    with tile.TileContext(nc) as tc:
        with (
            tc.tile_pool(name="big", bufs=1) as big,
            tc.tile_pool(name="sig", bufs=4) as sigp,
            tc.tile_pool(name="psum", bufs=1, space="PSUM") as pp,
        ):
            # --- input DMAs: first-needed tensors first per queue.
            XPC = big.tile([64, 2 * PLSZ], BF16)
            nc.sync.dma_start(out=XPC[:, :], in_=xpc_d[:, :])
            XAB = {}
            xb_t = big.tile([96, PLSZ], BF16, tag="xab1", name="xb_t")
            nc.scalar.dma_start(out=xb_t[:, :], in_=xb_d[:, :])
            XAB[1] = xb_t
            xa_t = big.tile([96, PLSZ], BF16, tag="xab0", name="xa_t")
            nc.sync.dma_start(out=xa_t[:, :], in_=xa_d[:, :])
            XAB[0] = xa_t
            wallc = big.tile([96, 448], BF16)
            nc.gpsimd.dma_start(out=wallc[:, :], in_=wallc_d[:, :])

            M, E = {}, {}
            for s in (0, 1):
                M[s] = big.tile([128, 2, 16, 32], BF16, tag=f"m{s}", name=f"M{s}")
                E[s] = big.tile([128, 2, 16, 32], BF16, tag=f"e{s}", name=f"E{s}")
            # C1 rows land straight in the M tiles from HBM
            nc.gpsimd.dma_start(out=M[1][96:128, :, :, :], in_=ms1_d[:, :])
            nc.gpsimd.dma_start(out=M[0][96:128, :, :, :], in_=ms0_d[:, :])
            TE = {}
            for s, dram, eng in ((1, te1_d, nc.scalar), (0, te0_d, nc.gpsimd)):
                t = big.tile([128, 2, 16, 32], BF16, tag=f"te{s}", name=f"TE{s}")
                eng.dma_start(out=t[:, :, :, :], in_=dram[:, :])
                TE[s] = t
            walla = big.tile([128, 320], BF16)
            nc.scalar.dma_start(out=walla[:, :], in_=walla_d[:, :])
            bias_t = big.tile([COUT, 1], F32)
            nc.gpsimd.dma_start(out=bias_t[:, :], in_=bias_d[:, :])

            CH1B = big.tile([32, 2, 16, 32], BF16)  # C4^s1 scratch (base 0)
            A0T = big.tile([32, 2, 16, 32], BF16)
            out_sb = big.tile([COUT, 4, 16, 32], BF16)
            scratch = big.tile([1, 8], F32)

            def xview(k, s, b):
                """Both-grid (32,[2,16,32]) view of tap k at partition base b."""
                ki, kj = divmod(k, 3)
                assert (s + ki + kj) % 2 == 1, "chain/XPC taps live on plane 1"
                off = []
                for t in (0, 1):
                    m = ((s ^ t) + kj) // 2
                    off.append(m * PLSZ + (t + ki) * PLP)
                return _fview(XPC[b : b + 32, :], off[0],
                              [[off[1] - off[0], 2], [2 * PLP, 16], [1, 32]])

            def mg(s, g):
                return M[s][32 * g : 32 * g + 32, :, :, :]

            # --- DVE: 6 chain muls + A0^s1 + 2 wide extras.  M layout is
            # [C2@g0, C3@g1, C4@g2, C1@g3]; c2/c3 read base 0, c4 base 32.
            nc.vector.tensor_copy(scratch[0:1, 0:1], XPC[0:1, 0:1])
            ch1bv = CH1B[:, :, :, :]
            nc.vector.tensor_mul(mg(1, 0), xview(6, 1, 0), xview(8, 1, 0))
            nc.vector.tensor_mul(mg(1, 1), xview(4, 1, 0), M[1][0:32, :, :, :])
            nc.vector.tensor_mul(ch1bv, xview(2, 1, 32), M[1][32:64, :, :, :])
            nc.scalar.activation(mg(1, 2), ch1bv,       # C4^s1 -> M1[g2]
                                 mybir.ActivationFunctionType.Copy)
            # A_0^s1 = x0 * C4^s1
            nc.vector.tensor_mul(A0T[:, :, :, :], xview(0, 1, 0), ch1bv)
            # s0 chain: all direct
            nc.vector.tensor_mul(mg(0, 0), xview(5, 0, 0), xview(7, 0, 0))
            # E1 = TE1 * M1 (g0/g1 DVE, g2 ACT copy, g3 seed DMA)
            nc.vector.tensor_copy(scratch[0:1, 0:1], TE[1][0:1, 0:1, 0:1, 0:1])
            nc.vector.tensor_copy(scratch[0:1, 1:2], M[1][64:65, 0:1, 0:1, 0:1])
            nc.vector.tensor_copy(scratch[0:1, 2:3], M[1][96:97, 0:1, 0:1, 0:1])
            nc.vector.tensor_mul(E[1][:, :, :, :], TE[1][:, :, :, :], M[1][:, :, :, :])
            nc.vector.tensor_mul(mg(0, 1), xview(3, 0, 0), M[0][0:32, :, :, :])
            nc.vector.tensor_mul(mg(0, 2), xview(1, 0, 32), M[0][32:64, :, :, :])
            # E0 = TE0 * M0 (g0..g2 DVE-local, g3 seed DMA)
            nc.vector.tensor_copy(scratch[0:1, 0:1], TE[0][0:1, 0:1, 0:1, 0:1])
            nc.vector.tensor_copy(scratch[0:1, 1:2], M[0][96:97, 0:1, 0:1, 0:1])
            nc.vector.tensor_mul(E[0][:, :, :, :], TE[0][:, :, :, :], M[0][:, :, :, :])

            # --- matmuls ---
            def convgrid(kj, s, t):
                """(96, 16,32) K=96 conv rhs: kernel-column kj, grid t."""
                c = (s + kj) % 2
                m = ((s ^ t) + kj) // 2
                off = t * PLP + m
                return _fview(XAB[c][:, :], off, [[2 * PLP, 16], [1, 32]])

            psq = {}

            def emit_conv(s, t):
                ps = pp.tile([128, 16, 32], F32, tag=f"ps{s}{t}", name=f"ps{s}{t}")
                psq[(s, t)] = ps
                if s == 1:
                    # conv-only, M=64 into rows 64:128; aeg region started
                    # separately by the first aeg matmul (start=True there).
                    for kj in range(3):
                        nc.tensor.matmul(
                            ps[64:128, :, :],
                            wallc[:, 64 * kj : 64 * kj + 64],
                            convgrid(kj, s, t),
                            start=(kj == 0), stop=False,
                            skip_group_check=True,
                        )
                else:
                    # kj2 first with braw columns (A_8^s0), M=128, resets both
                    nc.tensor.matmul(
                        ps[:, :, :], wallc[:, 320:448], convgrid(2, s, t),
                        start=True, stop=False, skip_group_check=True,
                    )
                    for kj in (0, 1):
                        nc.tensor.matmul(
                            ps[64:128, :, :],
                            wallc[:, 192 + 64 * kj : 256 + 64 * kj],
                            convgrid(kj, s, t),
                            start=False, stop=False, skip_group_check=True,
                        )

            def emit_aeg(s, t, which, start, stop):
                ps = psq[(s, t)]
                if which == "m":
                    lh = walla[:, 64 * (2 * s) : 64 * (2 * s) + 64]
                    rh = M[s][:, t, :, :]
                elif which == "e":
                    lh = walla[:, 64 * (2 * s + 1) : 64 * (2 * s + 1) + 64]
                    rh = E[s][:, t, :, :]
                else:  # a0 (s=1 only)
                    lh = walla[0:32, 256:320]
                    rh = A0T[:, t, :, :]
                nc.tensor.matmul(
                    ps[0:64, :, :], lh, rh,
                    start=start, stop=stop, skip_group_check=True,
                )

            emit_conv(1, 0)
            emit_conv(1, 1)
            emit_aeg(1, 0, "m", True, False)
            emit_aeg(1, 1, "m", True, False)
            emit_aeg(1, 0, "e", False, False)
            emit_aeg(1, 1, "e", False, False)
            emit_aeg(1, 0, "a0", False, True)
            emit_aeg(1, 1, "a0", False, True)
            emit_conv(0, 0)
            emit_conv(0, 1)
            emit_aeg(0, 0, "m", False, False)
            emit_aeg(0, 1, "m", False, False)
            emit_aeg(0, 0, "e", False, True)
            emit_aeg(0, 1, "e", False, True)

            # --- epilogue: sigmoid(aeg) * (conv + bias), per-quadrant DMA ---
            def emit_epi(s, t, eng):
                ps = psq[(s, t)]
                sig = sigp.tile([64, 16, 32], F32)
                nc.scalar.activation(
                    sig[:, :, :], ps[0:64, :, :],
                    mybir.ActivationFunctionType.Sigmoid,
                )
                nc.vector.tensor_copy(scratch[0:1, 0:1], sig[0:1, 0:1, 0:1])
                b = 2 * s + t
                nc.vector.scalar_tensor_tensor(
                    out=out_sb[:, b, :, :],
                    in0=ps[64:128, :, :],
                    scalar=bias_t[:, 0:1],
                    in1=sig[:, :, :],
                    op0=mybir.AluOpType.add,
                    op1=mybir.AluOpType.mult,
                )
                eng.dma_start(
                    out=out_d[b, :, :],
                    in_=out_sb[:, b, :, :],
                )

            emit_epi(1, 0, nc.sync)
            emit_epi(1, 1, nc.scalar)
            emit_epi(0, 0, nc.sync)
            emit_epi(0, 1, nc.scalar)
    nc.finalize()
    return nc


def _host_prep(x, weight, conv_w, conv_b):
    """Shard + pack per-core inputs (bf16 parity planes + weight products)."""
    bf16 = ml_dtypes.bfloat16
    xp = np.pad(np.ascontiguousarray(x, np.float32),
                ((0, 0), (0, 0), (PAD, PAD), (PAD, PAD)))
    kflat = weight.reshape(COUT, CIN, 9).transpose(2, 0, 1)  # (9, cout, cin)
    B = np.zeros((2, 9, COUT, CIN), np.float32)
    for s in (0, 1):
        suf = np.ones((COUT, CIN), np.float32)
        for k in range(8, -1, -1):
            B[s, k] = kflat[k] * suf
            if k % 2 == s:
                suf = suf * kflat[k]
    wc_k = conv_w.reshape(COUT, CIN, 9)  # (cout, cin, k)

    # conv lhsT [96, 448]: s1 kj0..2 (M=64) | s0 kj0, kj1 (M=64) |
    # s0 kj2 [braw | conv] (M=128)
    wallc = np.zeros((96, 448), np.float32)
    for kj in range(3):
        for ki in range(3):
            k = ki * 3 + kj
            blk = slice(32 * ki, 32 * ki + 32)
            wallc[blk, 64 * kj : 64 * kj + 64] = wc_k[:, :, k].T          # s1
            if kj < 2:
                wallc[blk, 192 + 64 * kj : 256 + 64 * kj] = wc_k[:, :, k].T
            else:
                wallc[blk, 384:448] = wc_k[:, :, k].T
    wallc[64:96, 320:384] = B[0, 8].T  # braw: A_8^s0 on the kj2 rhs rows

    # aeg lhsT: bM0 | bE0 | bM1 | bE1 | bA0
    walla = np.zeros((128, 320), np.float32)
    for s in (0, 1):
        for g, k in enumerate(M_TAPS[s]):
            walla[32 * g : 32 * g + 32, 64 * (2 * s) : 64 * (2 * s) + 64] = B[s, k].T
        for g, k in enumerate(E_TAPS[s]):
            walla[32 * g : 32 * g + 32,
                  64 * (2 * s + 1) : 64 * (2 * s + 1) + 64] = B[s, k].T
    walla[0:32, 256:320] = B[1, 0].T

    wallc_p = wallc.astype(bf16)
    walla_p = walla.astype(bf16)
    bias_p = np.ascontiguousarray(conv_b.reshape(COUT, 1), np.float32)

    in_maps = []
    for core in range(N_CORES):
        n, h = divmod(core, 2)
        slab = xp[n, :, 32 * h : 32 * h + ROWS, :]  # (32, 34, 66) f32
        plane1 = np.zeros((CIN, ROWS, PLP), np.float32)
        for r in range(ROWS):
            b = (1 + r) % 2
            cols = slab[:, r, b::2]
            plane1[:, r, : cols.shape[1]] = cols
        plane0 = np.zeros((CIN, ROWS, PLP), np.float32)
        for r in range(ROWS):
            b = r % 2
            cols = slab[:, r, b::2]
            plane0[:, r, : cols.shape[1]] = cols
        planes = {0: plane0, 1: plane1}
        # XPC: [plane1 | plane1-shifted], duplicated to partition base 32
        xpc = np.zeros((CIN, 2, ROWS, PLP), np.float32)
        xpc[:, 0] = plane1
        xpc[:, 1, :, : PLP - 1] = plane1[:, :, 1:]
        xpc_core = np.ascontiguousarray(
            np.tile(xpc.reshape(CIN, 2 * PLSZ), (2, 1))
        ).astype(bf16)
        # C1 seed rows (x7^s0 / x8^s1) in E-grid layout
        ms = np.zeros((2, CIN, 2, 16, 32), np.float32)
        for s in (0, 1):
            ki, kj = (2, 1) if s == 0 else (2, 2)
            for t in (0, 1):
                ms[s, :, t] = slab[:, t + ki : t + ki + 32 : 2,
                                   (s ^ t) + kj : (s ^ t) + kj + 64 : 2]
        ms0_core = np.ascontiguousarray(ms[0].reshape(32, 1024)).astype(bf16)
        ms1_core = np.ascontiguousarray(ms[1].reshape(32, 1024)).astype(bf16)
        # xa/xb: partition-stacked row-shifted plane sets for conv rhs
        xab = np.zeros((2, 3, CIN, ROWS, PLP), np.float32)
        for c in (0, 1):
            for r in range(3):
                q = (c + r) % 2
                xab[c, r, :, : ROWS - r] = planes[q][:, r:]
        xa_core = np.ascontiguousarray(xab[0].reshape(96, PLSZ)).astype(bf16)
        xb_core = np.ascontiguousarray(xab[1].reshape(96, PLSZ)).astype(bf16)
        # TE tap stacks (tight grid-major (2,16,32) per tap)
        te = np.zeros((2, 4, CIN, 2, 16, 32), np.float32)
        for s in (0, 1):
            for g, k in enumerate(E_TAPS[s]):
                ki, kj = divmod(k, 3)
                for t in (0, 1):
                    te[s, g, :, t] = slab[:, t + ki : t + ki + 32 : 2,
                                          (s ^ t) + kj : (s ^ t) + kj + 64 : 2]
        te0_core = np.ascontiguousarray(te[0].reshape(128, 1024)).astype(bf16)
        te1_core = np.ascontiguousarray(te[1].reshape(128, 1024)).astype(bf16)
        in_maps.append({
            "xpc": xpc_core,
            "xa": xa_core,
            "xb": xb_core,
            "te0": te0_core,
            "te1": te1_core,
            "ms0": ms0_core,
            "ms1": ms1_core,
            "wallc": wallc_p,
            "walla": walla_p,
            "bias": bias_p,
        })
    return in_maps


_nc_cache = None


def kernel(x, weight, conv_w, conv_b, trace=False):
    global _nc_cache, _last_results
    x = np.asarray(x, np.float32)
    weight = np.asarray(weight, np.float32)
    conv_w = np.asarray(conv_w, np.float32)
    conv_b = np.asarray(conv_b, np.float32)

    if _nc_cache is None:
        _nc_cache = build_nc()
    nc = _nc_cache
    in_maps = _host_prep(x, weight, conv_w, conv_b)
    res = run_bass_kernel_spmd(nc, in_maps, core_ids=list(range(N_CORES)), trace=trace)
    _last_results = res

    out = np.empty((N, COUT, H, W), np.float32)
    for core in range(N_CORES):
        n, h = divmod(core, 2)
        blk = res.results[core]["out"].astype(np.float32).reshape(2, 2, COUT, 16, 32)
        for s in (0, 1):
            for t in (0, 1):
                out[n, :, 32 * h + t : 32 * h + t + 32 : 2,
                    (s ^ t) :: 2] = blk[s, t]
    return out


# revision 18
# speedup vs baseline: 1.1295x; 1.0495x over previous
"""Trainium2 Bass kernel for nn_AEGConv2d (8 NeuronCores, SPMD).

Problem: out = sigmoid(aeg(x, weight)) * (conv2d(x, conv_w) + conv_b)
  x: (4, 32, 64, 64) f32, weight/conv_w: (64, 32, 3, 3), conv_b: (64,)
  stride=1, padding=1.

The AEG recurrence unrolls to res = sum_k A_k(px) * B_k(cout,cin) per
pixel-parity class s=(i+j)%2, where A_k = x_k * C_{sigma(s,k)} with the
suffix chain C_L over the opposite-class taps, and B_k a host-side
weight product.  The whole AEG conv is a 288-deep matmul per parity.

Sharding: 8 cores = 4 images x 2 row-halves.  No collectives.

Per-core device schedule (v3):
- XPC [plane1 | plane1-shifted]: every chain tap is a single both-grid
  32-partition view.  Chain muls write into the 128-partition multiplier
  stacks M_s = [C2, C3, C4, C1] (C2/C4 direct, C3/C4^s1 via a base-0
  scratch + off-path Pool/ACT copy, C1 an off-path raw-tap copy), which
  double as (a) in1 of one wide 128p "extras" mul per parity with the
  host-packed tap stacks TE0/TE1 and (b) a K=128 aeg-matmul rhs:
  M0 rows = [A_5,A_3,A_1,A_7], M1 = [A_6,A_4,A_2,A_8],
  E0 = [A_4,A_2,A_0,A_6], E1 = [A_5,A_3,A_1,A_7].
- DVE total: 9 muls + 4 epilogue STTs.  A_8^s0 rides the conv matmul
  (braw columns); A_0^s1 gets its own 32p mul + K=32 matmul.
- Matmuls: per quadrant (s,t) 3x conv K=96 + 2-3 aeg; quadrants close
  s1-first and each (s,t) output block DMAs right after its STT.
"""

import numpy as np
import ml_dtypes

import concourse.bacc as bacc
import concourse.bass as bass
import concourse.mybir as mybir
import concourse.tile as tile
from concourse.bass_utils import run_bass_kernel_spmd

F32 = mybir.dt.float32
BF16 = mybir.dt.bfloat16

N, CIN, H, W = 4, 32, 64, 64
COUT, KK = 64, 3
PAD = 1
OH, OW = 32, 64          # per-core output rows x cols
ROWS, COLS = 34, 66      # per-core padded slab
PLP = 34                 # plane row pitch
PLSZ = PLP * ROWS        # 1156 elements per plane per cin
N_CORES = 8

# chain taps (suffix products of the opposite-parity class), low level first:
# C1^s0=x7, C2=x5*C1, C3=x3*C2, C4=x1*C3 ; s1: x8, x6, x4, x2
CHAIN = {0: [7, 5, 3, 1], 1: [8, 6, 4, 2]}
# M-stack group layout is [C2, C3, C4, C1]; row tap identities:
M_TAPS = {0: [5, 3, 1, 7], 1: [6, 4, 2, 8]}
# TE row order multiplies [C2, C3, C4, C1]:
E_TAPS = {0: [4, 2, 0, 6], 1: [5, 3, 1, 7]}

_last_results = None  # stash for test.py (exec_time_ns etc.)


def _fview(base_ap, off, dims):
    """View with the same partition dim as base_ap but custom free dims."""
    return bass.AP(
        tensor=base_ap.tensor,
        offset=base_ap.offset + off,
        ap=[base_ap.ap[0]] + dims,
    )


def build_nc():
    nc = bacc.Bacc(None, target_bir_lowering=False)
    # plane-1 pair [unshifted | col-shifted], duplicated so chain muls can
    # read tap views at partition base 0 or 32 to match their in1 M-slot
    # (2-input DVE ops require equal base partitions for both inputs).
    xpc_d = nc.declare_dram_parameter("xpc", [64, 2 * PLSZ], BF16, isOutput=False)
    xa_d = nc.declare_dram_parameter("xa", [96, PLSZ], BF16, isOutput=False)
    xb_d = nc.declare_dram_parameter("xb", [96, PLSZ], BF16, isOutput=False)
    te0_d = nc.declare_dram_parameter("te0", [128, 1024], BF16, isOutput=False)
    te1_d = nc.declare_dram_parameter("te1", [128, 1024], BF16, isOutput=False)
    # host-packed C1 rows (raw x7^s0 / x8^s1 in E-grid layout) DMA'd
    # straight into M[s][96:128]
    ms0_d = nc.declare_dram_parameter("ms0", [32, 1024], BF16, isOutput=False)
    ms1_d = nc.declare_dram_parameter("ms1", [32, 1024], BF16, isOutput=False)
    wallc_d = nc.declare_dram_parameter("wallc", [96, 448], BF16, isOutput=False)
    walla_d = nc.declare_dram_parameter("walla", [128, 320], BF16, isOutput=False)
    bias_d = nc.declare_dram_parameter("bias", [COUT, 1], F32, isOutput=False)
    out_d = nc.declare_dram_parameter("out", [4, COUT, 512], BF16, isOutput=True)

    with tile.TileContext(nc) as tc:
        with (
            tc.tile_pool(name="big", bufs=1) as big,
            tc.tile_pool(name="sig", bufs=4) as sigp,
            tc.tile_pool(name="psum", bufs=1, space="PSUM") as pp,
        ):
            # --- input DMAs: first-needed tensors first per queue.
            XPC = big.tile([64, 2 * PLSZ], BF16)
            nc.sync.dma_start(out=XPC[:, :], in_=xpc_d[:, :])
            XAB = {}
            xb_t = big.tile([96, PLSZ], BF16, tag="xab1", name="xb_t")
            nc.scalar.dma_start(out=xb_t[:, :], in_=xb_d[:, :])
            XAB[1] = xb_t
            xa_t = big.tile([96, PLSZ], BF16, tag="xab0", name="xa_t")
            nc.sync.dma_start(out=xa_t[:, :], in_=xa_d[:, :])
            XAB[0] = xa_t
            wallc = big.tile([96, 448], BF16)
            nc.gpsimd.dma_start(out=wallc[:, :], in_=wallc_d[:, :])

            M, E = {}, {}
            for s in (0, 1):
                M[s] = big.tile([128, 2, 16, 32], BF16, tag=f"m{s}", name=f"M{s}")
                E[s] = big.tile([128, 2, 16, 32], BF16, tag=f"e{s}", name=f"E{s}")
            # C1 rows land straight in the M tiles from HBM
            nc.gpsimd.dma_start(out=M[1][96:128, :, :, :], in_=ms1_d[:, :])
            nc.gpsimd.dma_start(out=M[0][96:128, :, :, :], in_=ms0_d[:, :])
            TE = {}
            for s, dram, eng in ((1, te1_d, nc.scalar), (0, te0_d, nc.gpsimd)):
                t = big.tile([128, 2, 16, 32], BF16, tag=f"te{s}", name=f"TE{s}")
                eng.dma_start(out=t[:, :, :, :], in_=dram[:, :])
                TE[s] = t
            walla = big.tile([128, 320], BF16)
            nc.scalar.dma_start(out=walla[:, :], in_=walla_d[:, :])
            bias_t = big.tile([COUT, 1], F32)
            nc.gpsimd.dma_start(out=bias_t[:, :], in_=bias_d[:, :])

            CH1B = big.tile([32, 2, 16, 32], BF16)  # C4^s1 scratch (base 0)
            A0T = big.tile([32, 2, 16, 32], BF16)
            out_sb = big.tile([COUT, 4, 16, 32], BF16)
            scratch = big.tile([1, 8], F32)

            def xview(k, s, b):
                """Both-grid (32,[2,16,32]) view of tap k at partition base b."""
                ki, kj = divmod(k, 3)
                assert (s + ki + kj) % 2 == 1, "chain/XPC taps live on plane 1"
                off = []
                for t in (0, 1):
                    m = ((s ^ t) + kj) // 2
                    off.append(m * PLSZ + (t + ki) * PLP)
                return _fview(XPC[b : b + 32, :], off[0],
                              [[off[1] - off[0], 2], [2 * PLP, 16], [1, 32]])

            def mg(s, g):
                return M[s][32 * g : 32 * g + 32, :, :, :]

            # --- DVE: 6 chain muls + A0^s1 + 2 wide extras.  M layout is
            # [C2@g0, C3@g1, C4@g2, C1@g3]; c2/c3 read base 0, c4 base 32.
            nc.vector.tensor_copy(scratch[0:1, 0:1], XPC[0:1, 0:1])
            ch1bv = CH1B[:, :, :, :]
            nc.vector.tensor_mul(mg(1, 0), xview(6, 1, 0), xview(8, 1, 0))
            nc.vector.tensor_mul(mg(1, 1), xview(4, 1, 0), M[1][0:32, :, :, :])
            nc.vector.tensor_mul(ch1bv, xview(2, 1, 32), M[1][32:64, :, :, :])
            nc.scalar.activation(mg(1, 2), ch1bv,       # C4^s1 -> M1[g2]
                                 mybir.ActivationFunctionType.Copy)
            # A_0^s1 = x0 * C4^s1
            nc.vector.tensor_mul(A0T[:, :, :, :], xview(0, 1, 0), ch1bv)
            # s0 chain: all direct
            nc.vector.tensor_mul(mg(0, 0), xview(5, 0, 0), xview(7, 0, 0))
            # E1 = TE1 * M1 (g0/g1 DVE, g2 ACT copy, g3 seed DMA).  No touch
            # ops here: the tile scheduler hoists them ahead of the chains,
            # stalling the whole DVE stream on the TE DMAs.
            nc.vector.tensor_mul(E[1][:, :, :, :], TE[1][:, :, :, :], M[1][:, :, :, :])
            nc.vector.tensor_mul(mg(0, 1), xview(3, 0, 0), M[0][0:32, :, :, :])
            nc.vector.tensor_mul(mg(0, 2), xview(1, 0, 32), M[0][32:64, :, :, :])
            # E0 = TE0 * M0 (g0..g2 DVE-local, g3 seed DMA)
            nc.vector.tensor_mul(E[0][:, :, :, :], TE[0][:, :, :, :], M[0][:, :, :, :])

            # --- matmuls ---
            def convgrid(kj, s, t):
                """(96, 16,32) K=96 conv rhs: kernel-column kj, grid t."""
                c = (s + kj) % 2
                m = ((s ^ t) + kj) // 2
                off = t * PLP + m
                return _fview(XAB[c][:, :], off, [[2 * PLP, 16], [1, 32]])

            psq = {}

            def emit_conv(s, t):
                ps = pp.tile([128, 16, 32], F32, tag=f"ps{s}{t}", name=f"ps{s}{t}")
                psq[(s, t)] = ps
                if s == 1:
                    # conv-only, M=64 into rows 64:128; aeg region started
                    # separately by the first aeg matmul (start=True there).
                    for kj in range(3):
                        nc.tensor.matmul(
                            ps[64:128, :, :],
                            wallc[:, 64 * kj : 64 * kj + 64],
                            convgrid(kj, s, t),
                            start=(kj == 0), stop=False,
                            skip_group_check=True,
                        )
                else:
                    # kj2 first with braw columns (A_8^s0), M=128, resets both
                    nc.tensor.matmul(
                        ps[:, :, :], wallc[:, 320:448], convgrid(2, s, t),
                        start=True, stop=False, skip_group_check=True,
                    )
                    for kj in (0, 1):
                        nc.tensor.matmul(
                            ps[64:128, :, :],
                            wallc[:, 192 + 64 * kj : 256 + 64 * kj],
                            convgrid(kj, s, t),
                            start=False, stop=False, skip_group_check=True,
                        )

            def emit_aeg(s, t, which, start, stop):
                ps = psq[(s, t)]
                if which == "m":
                    lh = walla[:, 64 * (2 * s) : 64 * (2 * s) + 64]
                    rh = M[s][:, t, :, :]
                elif which == "e":
                    lh = walla[:, 64 * (2 * s + 1) : 64 * (2 * s + 1) + 64]
                    rh = E[s][:, t, :, :]
                else:  # a0 (s=1 only)
                    lh = walla[0:32, 256:320]
                    rh = A0T[:, t, :, :]
                nc.tensor.matmul(
                    ps[0:64, :, :], lh, rh,
                    start=start, stop=stop, skip_group_check=True,
                )

            emit_conv(1, 0)
            emit_conv(1, 1)
            emit_aeg(1, 0, "m", True, False)
            emit_aeg(1, 1, "m", True, False)
            emit_aeg(1, 0, "e", False, False)
            emit_aeg(1, 1, "e", False, False)
            emit_aeg(1, 0, "a0", False, True)
            emit_aeg(1, 1, "a0", False, True)
            emit_conv(0, 0)
            emit_conv(0, 1)
            emit_aeg(0, 0, "m", False, False)
            emit_aeg(0, 1, "m", False, False)
            emit_aeg(0, 0, "e", False, True)
            emit_aeg(0, 1, "e", False, True)

            # --- epilogue: sigmoid(aeg) * (conv + bias), per-quadrant DMA ---
            def emit_epi(s, t, eng):
                ps = psq[(s, t)]
                sig = sigp.tile([64, 16, 32], F32)
                nc.scalar.activation(
                    sig[:, :, :], ps[0:64, :, :],
                    mybir.ActivationFunctionType.Sigmoid,
                )
                nc.vector.tensor_copy(scratch[0:1, 0:1], sig[0:1, 0:1, 0:1])
                b = 2 * s + t
                nc.vector.scalar_tensor_tensor(
                    out=out_sb[:, b, :, :],
                    in0=ps[64:128, :, :],
                    scalar=bias_t[:, 0:1],
                    in1=sig[:, :, :],
                    op0=mybir.AluOpType.add,
                    op1=mybir.AluOpType.mult,
                )
                eng.dma_start(
                    out=out_d[b, :, :],
                    in_=out_sb[:, b, :, :],
                )

            emit_epi(1, 0, nc.sync)
            emit_epi(1, 1, nc.scalar)
            emit_epi(0, 0, nc.sync)
            emit_epi(0, 1, nc.scalar)
    nc.finalize()
    return nc


def _host_prep(x, weight, conv_w, conv_b):
    """Shard + pack per-core inputs (bf16 parity planes + weight products)."""
    bf16 = ml_dtypes.bfloat16
    xp = np.pad(np.ascontiguousarray(x, np.float32),
                ((0, 0), (0, 0), (PAD, PAD), (PAD, PAD)))
    kflat = weight.reshape(COUT, CIN, 9).transpose(2, 0, 1)  # (9, cout, cin)
    B = np.zeros((2, 9, COUT, CIN), np.float32)
    for s in (0, 1):
        suf = np.ones((COUT, CIN), np.float32)
        for k in range(8, -1, -1):
            B[s, k] = kflat[k] * suf
            if k % 2 == s:
                suf = suf * kflat[k]
    wc_k = conv_w.reshape(COUT, CIN, 9)  # (cout, cin, k)

    # conv lhsT [96, 448]: s1 kj0..2 (M=64) | s0 kj0, kj1 (M=64) |
    # s0 kj2 [braw | conv] (M=128)
    wallc = np.zeros((96, 448), np.float32)
    for kj in range(3):
        for ki in range(3):
            k = ki * 3 + kj
            blk = slice(32 * ki, 32 * ki + 32)
            wallc[blk, 64 * kj : 64 * kj + 64] = wc_k[:, :, k].T          # s1
            if kj < 2:
                wallc[blk, 192 + 64 * kj : 256 + 64 * kj] = wc_k[:, :, k].T
            else:
                wallc[blk, 384:448] = wc_k[:, :, k].T
    wallc[64:96, 320:384] = B[0, 8].T  # braw: A_8^s0 on the kj2 rhs rows

    # aeg lhsT: bM0 | bE0 | bM1 | bE1 | bA0
    walla = np.zeros((128, 320), np.float32)
    for s in (0, 1):
        for g, k in enumerate(M_TAPS[s]):
            walla[32 * g : 32 * g + 32, 64 * (2 * s) : 64 * (2 * s) + 64] = B[s, k].T
        for g, k in enumerate(E_TAPS[s]):
            walla[32 * g : 32 * g + 32,
                  64 * (2 * s + 1) : 64 * (2 * s + 1) + 64] = B[s, k].T
    walla[0:32, 256:320] = B[1, 0].T

    wallc_p = wallc.astype(bf16)
    walla_p = walla.astype(bf16)
    bias_p = np.ascontiguousarray(conv_b.reshape(COUT, 1), np.float32)

    in_maps = []
    for core in range(N_CORES):
        n, h = divmod(core, 2)
        slab = xp[n, :, 32 * h : 32 * h + ROWS, :]  # (32, 34, 66) f32
        plane1 = np.zeros((CIN, ROWS, PLP), np.float32)
        for r in range(ROWS):
            b = (1 + r) % 2
            cols = slab[:, r, b::2]
            plane1[:, r, : cols.shape[1]] = cols
        plane0 = np.zeros((CIN, ROWS, PLP), np.float32)
        for r in range(ROWS):
            b = r % 2
            cols = slab[:, r, b::2]
            plane0[:, r, : cols.shape[1]] = cols
        planes = {0: plane0, 1: plane1}
        # XPC: [plane1 | plane1-shifted], duplicated to partition base 32
        xpc = np.zeros((CIN, 2, ROWS, PLP), np.float32)
        xpc[:, 0] = plane1
        xpc[:, 1, :, : PLP - 1] = plane1[:, :, 1:]
        xpc_core = np.ascontiguousarray(
            np.tile(xpc.reshape(CIN, 2 * PLSZ), (2, 1))
        ).astype(bf16)
        # C1 seed rows (x7^s0 / x8^s1) in E-grid layout
        ms = np.zeros((2, CIN, 2, 16, 32), np.float32)
        for s in (0, 1):
            ki, kj = (2, 1) if s == 0 else (2, 2)
            for t in (0, 1):
                ms[s, :, t] = slab[:, t + ki : t + ki + 32 : 2,
                                   (s ^ t) + kj : (s ^ t) + kj + 64 : 2]
        ms0_core = np.ascontiguousarray(ms[0].reshape(32, 1024)).astype(bf16)
        ms1_core = np.ascontiguousarray(ms[1].reshape(32, 1024)).astype(bf16)
        # xa/xb: partition-stacked row-shifted plane sets for conv rhs
        xab = np.zeros((2, 3, CIN, ROWS, PLP), np.float32)
        for c in (0, 1):
            for r in range(3):
                q = (c + r) % 2
                xab[c, r, :, : ROWS - r] = planes[q][:, r:]
        xa_core = np.ascontiguousarray(xab[0].reshape(96, PLSZ)).astype(bf16)
        xb_core = np.ascontiguousarray(xab[1].reshape(96, PLSZ)).astype(bf16)
        # TE tap stacks (tight grid-major (2,16,32) per tap)
        te = np.zeros((2, 4, CIN, 2, 16, 32), np.float32)
        for s in (0, 1):
            for g, k in enumerate(E_TAPS[s]):
                ki, kj = divmod(k, 3)
                for t in (0, 1):
                    te[s, g, :, t] = slab[:, t + ki : t + ki + 32 : 2,
                                          (s ^ t) + kj : (s ^ t) + kj + 64 : 2]
        te0_core = np.ascontiguousarray(te[0].reshape(128, 1024)).astype(bf16)
        te1_core = np.ascontiguousarray(te[1].reshape(128, 1024)).astype(bf16)
        in_maps.append({
            "xpc": xpc_core,
            "xa": xa_core,
            "xb": xb_core,
            "te0": te0_core,
            "te1": te1_core,
            "ms0": ms0_core,
            "ms1": ms1_core,
            "wallc": wallc_p,
            "walla": walla_p,
            "bias": bias_p,
        })
    return in_maps


_nc_cache = None


def kernel(x, weight, conv_w, conv_b, trace=False):
    global _nc_cache, _last_results
    x = np.asarray(x, np.float32)
    weight = np.asarray(weight, np.float32)
    conv_w = np.asarray(conv_w, np.float32)
    conv_b = np.asarray(conv_b, np.float32)

    if _nc_cache is None:
        _nc_cache = build_nc()
    nc = _nc_cache
    in_maps = _host_prep(x, weight, conv_w, conv_b)
    res = run_bass_kernel_spmd(nc, in_maps, core_ids=list(range(N_CORES)), trace=trace)
    _last_results = res

    out = np.empty((N, COUT, H, W), np.float32)
    for core in range(N_CORES):
        n, h = divmod(core, 2)
        blk = res.results[core]["out"].astype(np.float32).reshape(2, 2, COUT, 16, 32)
        for s in (0, 1):
            for t in (0, 1):
                out[n, :, 32 * h + t : 32 * h + t + 32 : 2,
                    (s ^ t) :: 2] = blk[s, t]
    return out
